# revision 55
# baseline (speedup 1.0000x reference)
"""Multi-head causal attention (B=16, T=512, D=1024, H=16) on 8 TRN2 cores.

Sharding: data-parallel over batch (2 batches per core), weights replicated;
no collectives needed.

Per-core kernel (modeled 170.7us, rel err 1.6e-3 on HW; baseline 281us):
  - QKV projection fused over both local batches: each w_qkv k-tile is
    DMA'd ONCE and used by both batches (12MB instead of 24MB of weight
    traffic); f32r matmuls at full PE rate, x(b0) rides the SP DMA queue
    ahead of the weight stream, x(b1)/biases ride the Pool queue.
  - Attention in bf16 via the S^T scheme: S^T = K_h^T.T @ Q_h^T computed
    k-major so softmax needs NO P transposes; kt=2,3 chunks share one PSUM
    bank so exp is 3 fused ACT ops/head; causal masking is a post-exp
    affine_select (zero q<k) on the otherwise-idle GPSIMD engine.
  - AV uses P^T blocks as stationary against V tiles carrying a 65th
    all-ones column, so y arrives token-major WITH the softmax row sums in
    column 64; normalize = per-partition reciprocal + tensor_scalar_mul on
    the PSUM evac; head pairs are PE-transposed back to feature-major y^T
    (f32r) for the output projection. S-stages run two heads ahead of
    AV-stages to hide exp latency.
  - Biases: K bias dropped (softmax row-invariance, exact); Q bias folded
    into the PSUM evac (tensor_scalar_add / ACT Identity-bias); V and out
    biases are free-dim adds against one-time broadcast tiles on the evac
    (tensor_tensor), so no rank-1 bias matmuls remain on the PE.
  - Schedule: x transposes -> Q,K secs for both batches (pass b0 streams
    weight tiles once, pass b1 reuses) -> attention with 3-head S-stage
    lookahead; ALL V projection groups (b0 then b1) are PE filler inside
    attn(b0) so the ACT exp stream starts ~14us earlier; attn(b1) shares
    the same PSUM pools (no handoff) and interleaves the out-projection of
    b0 tokens; tail is the out-projection of b1 tokens with the final DMA
    split across both queues. x(b0) rides the SP DMA queue ahead of the
    weight stream, then x(b1) behind sec0 so weight arrival paces Q(b0);
    biases ride the Pool queue.
"""

import sys

sys.path.insert(0, "/opt/trn_rl_repo")

import numpy as np

B, T, D = 16, 512, 1024
H = 16
HD = D // H          # 64
NCORES = 8
BL = B // NCORES     # 2 local batches per core
PPART = 128

_CACHE = {}


def _build_program(reps=1, phases="xqaw"):
    import concourse.bass as bass
    import concourse.tile as tile
    from concourse import bacc, mybir
    from concourse.masks import make_identity

    DT = mybir.dt.float32
    R = mybir.dt.float32r
    BF = mybir.dt.bfloat16
    ACTF = mybir.ActivationFunctionType
    ALU = mybir.AluOpType

    nc = bacc.Bacc("TRN2", target_bir_lowering=False, debug=False,
                   num_devices=NCORES)

    x_d = nc.dram_tensor("x", [BL, T, D], DT, kind="ExternalInput").ap()
    wqkv_d = nc.dram_tensor("w_qkv", [D, 3 * D], DT, kind="ExternalInput").ap()
    bqkv_d = nc.dram_tensor("b_qkv", [3 * D], DT, kind="ExternalInput").ap()
    wo_d = nc.dram_tensor("w_o", [D, D], DT, kind="ExternalInput").ap()
    bo_d = nc.dram_tensor("b_o", [D], DT, kind="ExternalInput").ap()
    out_d = nc.dram_tensor("out", [BL, T, D], DT, kind="ExternalOutput").ap()

    x_f = x_d.flatten_outer_dims()      # [1024, 1024] tokens x features
    out_fs = [out_d.flatten_outer_dims()]
    for r in range(1, reps):
        scr = nc.dram_tensor(f"scratch{r}", [BL, T, D], DT).ap()
        out_fs.append(scr.flatten_outer_dims())

    def f32r(ap):
        return ap.bitcast(R)

    with tile.TileContext(nc) as tc:
        with (
            tc.tile_pool(name="consts", bufs=1) as consts,
            tc.tile_pool(name="y", bufs=1) as y_pool,
            tc.tile_pool(name="xt", bufs=2) as xt_pool,
            tc.tile_pool(name="qkv", bufs=1) as qkv_pool,
            tc.tile_pool(name="w", bufs=9) as w_pool,
            tc.tile_pool(name="xn", bufs=3) as xn_pool,
            tc.tile_pool(name="pp", bufs=12) as p_pool,
            tc.tile_pool(name="ytk", bufs=3) as ytk_pool,
            tc.tile_pool(name="ss", bufs=4) as s_pool,
            tc.tile_pool(name="ob", bufs=2) as o_pool,
        ):
            # ---------------- constants ----------------
            ident_f = consts.tile([PPART, PPART], DT)
            make_identity(nc, ident_f)
            ident = consts.tile([PPART, PPART], R)
            nc.vector.tensor_copy(out=ident, in_=ident_f)
            ident_b = consts.tile([PPART, PPART], BF)
            nc.vector.tensor_copy(out=ident_b, in_=ident_f)

            # mask for S^T [k-part, q-free]: keep q >= k, else -1e30
            maskneg = consts.tile([PPART, PPART], DT)
            nc.vector.memset(maskneg, 0.0)
            nc.gpsimd.affine_select(
                out=maskneg, in_=maskneg,
                compare_op=ALU.is_ge, fill=-1e30,
                base=0, pattern=[[1, PPART]], channel_multiplier=-1,
            )
            mask_bT = consts.tile([PPART, PPART], BF)
            nc.vector.tensor_copy(out=mask_bT, in_=maskneg)

            ones_f = consts.tile([1, PPART], DT)
            nc.vector.memset(ones_f, 1.0)
            ones_row = consts.tile([1, PPART], R)
            nc.vector.tensor_copy(out=ones_row, in_=ones_f)

            # biases: Q bias as [128, 8] columns (per-partition add on evac);
            # V bias as a [1, 1024] row (rank-1 matmul); output bias row.
            bq_col = consts.tile([PPART, 8], DT)
            bv_sb = consts.tile([1, D], R)
            bo_sb = consts.tile([1, D], R)
            bv_bc = consts.tile([PPART, D], BF)
            bo_bc = consts.tile([PPART, D], BF)

            def build_bias_bcast(qps):
                # broadcast V/O bias rows to all 128 partitions once, so the
                # per-group rank-1 bias matmuls disappear from the PE
                for dst, srow in ((bv_bc, bv_sb), (bo_bc, bo_sb)):
                    for half in range(2):
                        ps = qps.tile([PPART, T], DT, tag="ps", name="bbc")
                        nc.tensor.matmul(
                            ps, lhsT=ones_row,
                            rhs=srow[:, 512 * half:512 * (half + 1)],
                            start=True, stop=True)
                        nc.scalar.activation(
                            out=dst[:, 512 * half:512 * (half + 1)], in_=ps,
                            func=ACTF.Copy)

            def load_biases():
                nc.gpsimd.dma_start(
                    out=bq_col,
                    in_=bqkv_d.rearrange("(f p) -> p f", p=PPART)[:, 0:8])
                nc.gpsimd.dma_start(
                    out=bv_sb,
                    in_=f32r(bqkv_d.rearrange("(a f) -> a f", a=1))[:, 2 * D:3 * D])
                nc.gpsimd.dma_start(
                    out=bo_sb, in_=f32r(bo_d.rearrange("(a f) -> a f", a=1)))

            y_t = y_pool.tile([PPART, 8, BL * T], R)  # [128, 8, 1024] f32r

            def load_x(b, eng):
                xns = []
                for to in range(4):
                    xn = xn_pool.tile([PPART, D], R, tag="xn",
                                      name=f"xn{b}{to}")
                    rows = x_f[T * b + 128 * to:T * b + 128 * (to + 1), :]
                    if b == 0 and to == 0:
                        # split the very first load so fg0 transposes can
                        # start half a transfer earlier
                        eng.dma_start(out=xn[:, 0:512],
                                      in_=f32r(rows[:, 0:512]))
                        eng.dma_start(out=xn[:, 512:1024],
                                      in_=f32r(rows[:, 512:1024]))
                    else:
                        eng.dma_start(out=xn, in_=f32r(rows))
                    xns.append(xn)
                return xns

            def transpose_x(b, xns, trps, x_t):
                # to-major: each arriving dual-block is transposed (both
                # feature groups) while the next is still on the wire
                for to in range(4):
                    for fg in range(2):
                        pst = trps.tile([PPART, 4, PPART], R, tag="ps",
                                        name="pstx")
                        for fi in range(4):
                            fo = 4 * fg + fi
                            nc.tensor.transpose(
                                pst[:, fi, :],
                                xns[to][:, 128 * fo:128 * (fo + 1)],
                                ident)
                        if fg == 0:
                            nc.scalar.activation(
                                out=x_t[:, 4 * fg:4 * (fg + 1),
                                        128 * to:128 * (to + 1)],
                                in_=pst, func=ACTF.Copy)
                        else:
                            nc.vector.tensor_copy(
                                out=x_t[:, 4 * fg:4 * (fg + 1),
                                        128 * to:128 * (to + 1)],
                                in_=pst)

            # ---------------- QKV projection (fused over batches) ---------
            # q_t/k_t: [128(2 heads of pair j), pair j, T] bf16 feature-major
            # v_t: [128 tok, to, head, 65] bf16 token-major (+ones col)
            def stream_w_sec(sec):
                # quad-tiles: 2 DMAs per sec instead of 8 (SWDGE descriptor
                # generation on the queue is the scarce resource, not BW)
                wt = []
                for ko in range(8):
                    w_sb = w_pool.tile([PPART, 1024], R, tag="w",
                                       name=f"w{sec}_{ko}")
                    nc.sync.dma_start(
                        out=w_sb,
                        in_=f32r(wqkv_d[128 * ko:128 * (ko + 1),
                                        1024 * sec:1024 * (sec + 1)]))
                    wt.append(w_sb)
                return wt

            def qk_pass(sec, b, wt, x_t, qps, dst):
                psums = [qps.tile([PPART, T], DT, tag="ps",
                                  name=f"qkps{i}") for i in range(8)]
                for ko in range(8):
                    for fo in range(8):
                        nc.tensor.matmul(
                            psums[fo],
                            lhsT=wt[ko][:, 128 * fo:128 * (fo + 1)],
                            rhs=x_t[:, ko, :],
                            start=(ko == 0), stop=(ko == 7))
                for fo in range(8):
                    if sec == 0:  # Q: add bias on evac (per-partition)
                        if fo % 2 == 0:
                            nc.scalar.activation(
                                out=dst[:, fo, :], in_=psums[fo],
                                func=ACTF.Identity,
                                bias=bq_col[:, fo:fo + 1])
                        else:
                            nc.vector.tensor_scalar_add(
                                out=dst[:, fo, :], in0=psums[fo],
                                scalar1=bq_col[:, fo:fo + 1])
                    else:         # K: bias dropped (softmax-invariant)
                        if fo % 2 == 0:
                            nc.scalar.activation(
                                out=dst[:, fo, :], in_=psums[fo],
                                func=ACTF.Copy)
                        else:
                            nc.vector.tensor_copy(
                                out=dst[:, fo, :], in_=psums[fo])

            wv_tiles = []

            def v_group(b, g, x_ts, v_ts, pool, tag="vps", evac="act"):
                to, nh = g // 2, g % 2
                ps = pool.tile([PPART, T], DT, tag=tag, name="vps")
                for ko in range(8):
                    nc.tensor.matmul(
                        ps,
                        lhsT=x_ts[b][:, ko, 128 * to:128 * (to + 1)],
                        rhs=wv_tiles[ko][:, 512 * nh:512 * (nh + 1)],
                        start=(ko == 0), stop=(ko == 7))
                nc.vector.tensor_add(
                    out=v_ts[b][:, to, 8 * nh:8 * (nh + 1), 0:64],
                    in0=ps, in1=bv_bc[:, 512 * nh:512 * (nh + 1)])

            # ---------------- attention (S^T scheme, bf16) ----------------
            def s_stage(b, h, q_ts, k_ts, sps):
                # kt=2 (256 cols) and kt=3 (128 cols) share one PSUM bank and
                # one fused exp op; returns [pch01 (kt0), pch1 (kt1), pch23]
                j, r = h // 2, h % 2
                base = 64 * r
                pchunks = []
                offs = [0, 0, 0, 256]   # col offset of chunk kt in its tile
                tiles = {}
                for kt in range(4):
                    cols = (4 - kt) * 128
                    if kt == 3:
                        ps = tiles[2]
                    else:
                        ps = sps.tile([PPART, T], DT, tag="s", name="sps")
                        tiles[kt] = ps
                    o = offs[kt]
                    nc.tensor.matmul(
                        ps[:, o:o + cols],
                        lhsT=k_ts[b][base:base + 64, j,
                                     128 * kt:128 * (kt + 1)],
                        rhs=q_ts[b][base:base + 64, j, 128 * kt:],
                        start=True, stop=True)
                for kt, cols in ((0, 512), (1, 384), (2, 384)):
                    pch = p_pool.tile([PPART, T], BF, tag="P", name="pch")
                    nc.scalar.activation(
                        out=pch[:, :cols], in_=tiles[kt][:, :cols],
                        func=ACTF.Exp, scale=1.0 / 32.0)
                    # causal mask: zero q < k on this chunk's diagonal
                    # block(s) on the (otherwise idle) GPSIMD engine
                    nc.gpsimd.affine_select(
                        out=pch[:, 0:128], in_=pch[:, 0:128],
                        compare_op=ALU.is_ge, fill=0.0,
                        base=0, pattern=[[1, PPART]], channel_multiplier=-1)
                    if kt == 2:  # kt=3's diagonal lives at offset 256
                        nc.gpsimd.affine_select(
                            out=pch[:, 256:384], in_=pch[:, 256:384],
                            compare_op=ALU.is_ge, fill=0.0,
                            base=0, pattern=[[1, PPART]],
                            channel_multiplier=-1)
                    pchunks.append(pch)
                return pchunks

            pair_state = {}

            def av_stage(b, h, pchunks, v_ts, yps, trps):
                j, r = h // 2, h % 2
                yp = yps.tile([PPART, 4, 65], DT, tag="y")
                for qt in range(4):
                    for kt in range(qt + 1):
                        pch = pchunks[min(kt, 2)]
                        o = 256 if kt == 3 else 0
                        nc.tensor.matmul(
                            yp[:, qt, :],
                            lhsT=pch[:, o + 128 * (qt - kt):
                                     o + 128 * (qt - kt + 1)],
                            rhs=v_ts[b][:, kt, h, :],
                            start=(kt == 0), stop=(kt == qt))
                rs = s_pool.tile([PPART, 4, 1], DT, tag="rs")
                nc.vector.reciprocal(rs, yp[:, :, 64:65])
                if r == 0:
                    ytk = ytk_pool.tile([PPART, 4, PPART], BF, tag="ytk")
                    pair_state[0] = ytk
                else:
                    ytk = pair_state[0]
                for qt in range(4):
                    nc.vector.tensor_scalar_mul(
                        out=ytk[:, qt, 64 * r:64 * (r + 1)],
                        in0=yp[:, qt, 0:64], scalar1=rs[:, qt, :])
                if r == 1:  # pair complete: transpose back to feature-major
                    pst = trps.tile([PPART, 4, PPART], BF, tag="ytr")
                    for to in range(4):
                        nc.tensor.transpose(
                            pst[:, to, :], ytk[:, to, :], ident_b)
                    if b == 0 and j < 4:
                        nc.scalar.activation(
                            out=y_t[:, j, T * b:T * (b + 1)],
                            in_=pst, func=ACTF.Copy)
                    else:
                        nc.vector.tensor_copy(
                            out=y_t[:, j, T * b:T * (b + 1)], in_=pst)

            wo_tiles = {}

            def load_wo(wo_pool):
                for ko in range(8):
                    w_sb = wo_pool.tile([PPART, 1024], R, tag="w",
                                        name=f"wo{ko}")
                    nc.sync.dma_start(
                        out=w_sb, in_=f32r(wo_d[128 * ko:128 * (ko + 1), :]))
                    wo_tiles[ko] = w_sb

            wo_state = {}

            def wo_half(tg, nh, wps, out_f, split_dma=False):
                # both nh halves of a token group share one [128, 1024] evac
                # buffer and a single contiguous output DMA (issued on nh=1)
                if nh == 0:
                    ob = o_pool.tile([PPART, D], DT, tag="ob", name="ob")
                    wo_state[tg] = ob
                else:
                    ob = wo_state[tg]
                ps = wps.tile([PPART, T], DT, tag="s", name="wops")
                for ko in range(8):
                    nc.tensor.matmul(
                        ps,
                        lhsT=y_t[:, ko, 128 * tg:128 * (tg + 1)],
                        rhs=wo_tiles[ko][:, 512 * nh:512 * (nh + 1)],
                        start=(ko == 0), stop=(ko == 7))
                nc.vector.tensor_add(
                    out=ob[:, 512 * nh:512 * (nh + 1)], in0=ps,
                    in1=bo_bc[:, 512 * nh:512 * (nh + 1)])
                if nh == 1:
                    if split_dma:
                        for s in range(2):
                            q = nc.sync if s == 0 else nc.gpsimd
                            q.dma_start(
                                out=out_f[128 * tg:128 * (tg + 1),
                                          512 * s:512 * (s + 1)],
                                in_=ob[:, 512 * s:512 * (s + 1)])
                    else:
                        q = nc.sync if tg % 2 == 0 else nc.gpsimd
                        q.dma_start(
                            out=out_f[128 * tg:128 * (tg + 1), :], in_=ob)

            # ---------------- schedule ----------------
            for rep in range(reps):
                out_f = out_fs[rep]
                sfx = str(rep)
                wv_tiles.clear()
                wo_tiles.clear()

                x_t0 = xt_pool.tile([PPART, 8, T], R, tag="xt", name="xt0")
                x_t1 = xt_pool.tile([PPART, 8, T], R, tag="xt", name="xt1")
                x_ts = [x_t0, x_t1]

                q_ts = [qkv_pool.tile([PPART, 8, T], BF, tag=f"q{b}",
                                       name=f"q{b}")
                        for b in range(2)]
                k_ts = [qkv_pool.tile([PPART, 8, T], BF, tag=f"k{b}",
                                       name=f"k{b}")
                        for b in range(2)]
                v_ts = [qkv_pool.tile([PPART, 4, H, 65], BF, tag=f"v{b}",
                                       name=f"v{b}")
                        for b in range(2)]

                with tc.tile_pool(name="qps" + sfx, bufs=8,
                                  space="PSUM") as qps:
                    # x(b0) first on the fast SP queue, weights behind it;
                    # x(b1) + biases ride the Pool queue concurrently
                    xns0 = load_x(0, nc.sync)
                    wt0 = stream_w_sec(0)
                    if rep == 0:
                        load_biases()
                    xns1 = load_x(1, nc.sync)
                    transpose_x(0, xns0, qps, x_t0)
                    build_bias_bcast(qps)
                    qk_pass(0, 0, wt0, x_t0, qps, q_ts[0])
                    transpose_x(1, xns1, qps, x_t1)
                    wt1 = stream_w_sec(1)
                    qk_pass(0, 1, wt0, x_t1, qps, q_ts[1])
                    qk_pass(1, 0, wt1, x_t0, qps, k_ts[0])
                    wv_tiles.extend(stream_w_sec(2))
                    qk_pass(1, 1, wt1, x_t1, qps, k_ts[1])
                    for b in range(2):
                        nc.vector.memset(v_ts[b][:, :, :, 64:65], 1.0)

                # attn(b0): 2-head S-stage lookahead hides exp latency; ALL
                # V projection groups (b0 then b1) fill the PE between AV
                # stages, so attention (and its ACT exp stream) starts 14us
                # earlier. Groups a head's AV consumes must be emitted
                # before that AV (PE executes in order): nh=0 groups before
                # AV(0), nh=1 groups before AV(8).
                with (
                    tc.tile_pool(name="as0" + sfx, bufs=4, space="PSUM") as sps,
                    tc.tile_pool(name="ay0" + sfx, bufs=2, space="PSUM") as yps,
                    tc.tile_pool(name="at0" + sfx, bufs=2, space="PSUM") as trps,
                ):
                    order = ([(0, g) for g in (1, 3, 5, 7)]
                             + [(1, g) for g in range(8)])
                    vg = iter(order)
                    pend = []
                    for h in range(16):
                        pend.append((h, s_stage(0, h, q_ts, k_ts, sps)))
                        if h < 2:  # nh=0 groups of b0 up front
                            v_group(0, 4 * h + 0, x_ts, v_ts, sps, tag="s",
                                    evac="dve")
                            v_group(0, 4 * h + 2, x_ts, v_ts, sps, tag="s",
                                    evac="dve")
                        if len(pend) == 4:
                            hh, pch = pend.pop(0)
                            av_stage(0, hh, pch, v_ts, yps, trps)
                            bg = next(vg, None)
                            if bg is not None:
                                v_group(bg[0], bg[1], x_ts, v_ts, sps,
                                        tag="s", evac="dve")
                            if hh == 9:
                                load_wo(w_pool)
                    for hh, pch in pend:
                        av_stage(0, hh, pch, v_ts, yps, trps)
                        bg = next(vg, None)
                        if bg is not None:
                            v_group(bg[0], bg[1], x_ts, v_ts, sps,
                                    tag="s", evac="dve")

                    # attn(b1) interleaved with out-projection of b0 tokens
                    # (same pools: no PSUM handoff between the phases)
                    pend = []
                    wo_m = 0
                    for h in range(16):
                        pend.append((h, s_stage(1, h, q_ts, k_ts, sps)))
                        if len(pend) == 4:
                            hh, pch = pend.pop(0)
                            av_stage(1, hh, pch, v_ts, yps, trps)
                            if hh % 2 == 1:
                                wo_half(wo_m // 2, wo_m % 2, sps, out_f)
                                wo_m += 1
                    for hh, pch in pend:
                        av_stage(1, hh, pch, v_ts, yps, trps)
                    while wo_m < 8:
                        wo_half(wo_m // 2, wo_m % 2, sps, out_f)
                        wo_m += 1
                    # out-projection of b1 tokens; the last DMA is split
                    # across both queues so the tail drains faster
                    for tg in range(4, 8):
                        for nh in range(2):
                            wo_half(tg, nh, sps, out_f, split_dma=(tg == 7))

    nc.compile()
    return nc


def _get_program(reps=1, phases="xqaw"):
    key = f"nc{reps}{phases}"
    if key not in _CACHE:
        _CACHE[key] = _build_program(reps, phases)
    return _CACHE[key]


def kernel(x, w_qkv, b_qkv, w_o, b_o):
    from concourse.bass_utils import run_bass_kernel_spmd

    nc = _get_program()
    x = np.ascontiguousarray(x, dtype=np.float32)
    in_maps = []
    for c in range(NCORES):
        in_maps.append({
            "x": x[BL * c:BL * (c + 1)],
            "w_qkv": np.asarray(w_qkv, dtype=np.float32),
            "b_qkv": np.asarray(b_qkv, dtype=np.float32),
            "w_o": np.asarray(w_o, dtype=np.float32),
            "b_o": np.asarray(b_o, dtype=np.float32),
        })
    res = run_bass_kernel_spmd(nc, in_maps, list(range(NCORES)))
    return np.concatenate([res.results[c]["out"] for c in range(NCORES)], axis=0)


# revision 61
# speedup vs baseline: 1.1372x; 1.1372x over previous
"""Multi-head causal attention (B=16, T=512, D=1024, H=16) on 8 TRN2 cores.

Sharding: data-parallel over batch (2 batches per core), weights replicated;
no collectives needed.

Per-core kernel (modeled 169.0us, rel err 1.6e-3 on HW; baseline 281us):
  - QKV projection fused over both local batches: each w_qkv k-tile is
    DMA'd ONCE and used by both batches (12MB instead of 24MB of weight
    traffic); f32r matmuls at full PE rate, x(b0) rides the SP DMA queue
    ahead of the weight stream, x(b1)/biases ride the Pool queue.
  - Attention in bf16 via the S^T scheme: S^T = K_h^T.T @ Q_h^T computed
    k-major so softmax needs NO P transposes; kt=2,3 chunks share one PSUM
    bank so exp is 3 fused ACT ops/head; causal masking is a post-exp
    affine_select (zero q<k) on the otherwise-idle GPSIMD engine.
  - AV uses P^T blocks as stationary against V tiles carrying a 65th
    all-ones column, so y arrives token-major WITH the softmax row sums in
    column 64; normalize = per-partition reciprocal + one fused broadcast
    tensor_mul on the PSUM evac; head pairs are PE-transposed back to
    feature-major y^T (f32r) for the output projection. S-stages run three
    heads ahead of AV-stages to hide exp latency.
  - Biases: K bias dropped (softmax row-invariance, exact); Q bias folded
    into the PSUM evac (tensor_scalar_add / ACT Identity-bias); V and out
    biases are free-dim adds against one-time broadcast tiles on the evac
    (tensor_tensor), so no rank-1 bias matmuls remain on the PE.
  - Schedule: x transposes -> Q,K secs for both batches (pass b0 streams
    weight tiles once, pass b1 reuses) -> attention with 3-head S-stage
    lookahead; ALL V projection groups (b0 then b1) are PE filler inside
    attn(b0) so the ACT exp stream starts ~14us earlier; attn(b1) shares
    the same PSUM pools (no handoff) and interleaves the out-projection of
    b0 tokens; tail is the out-projection of b1 tokens with the final DMA
    split across both queues. x(b0) rides the SP DMA queue ahead of the
    weight stream, then x(b1) behind sec0 so weight arrival paces Q(b0);
    biases ride the Pool queue.
"""

import sys

sys.path.insert(0, "/opt/trn_rl_repo")

import numpy as np

B, T, D = 16, 512, 1024
H = 16
HD = D // H          # 64
NCORES = 8
BL = B // NCORES     # 2 local batches per core
PPART = 128

_CACHE = {}


def _build_program(reps=1, phases="xqaw"):
    import concourse.bass as bass
    import concourse.tile as tile
    from concourse import bacc, mybir
    from concourse.masks import make_identity

    DT = mybir.dt.float32
    R = mybir.dt.float32r
    BF = mybir.dt.bfloat16
    ACTF = mybir.ActivationFunctionType
    ALU = mybir.AluOpType

    nc = bacc.Bacc("TRN2", target_bir_lowering=False, debug=False,
                   num_devices=NCORES)

    x_d = nc.dram_tensor("x", [BL, T, D], DT, kind="ExternalInput").ap()
    wqkv_d = nc.dram_tensor("w_qkv", [D, 3 * D], DT, kind="ExternalInput").ap()
    bqkv_d = nc.dram_tensor("b_qkv", [3 * D], DT, kind="ExternalInput").ap()
    wo_d = nc.dram_tensor("w_o", [D, D], DT, kind="ExternalInput").ap()
    bo_d = nc.dram_tensor("b_o", [D], DT, kind="ExternalInput").ap()
    out_d = nc.dram_tensor("out", [BL, T, D], DT, kind="ExternalOutput").ap()

    x_f = x_d.flatten_outer_dims()      # [1024, 1024] tokens x features
    out_fs = [out_d.flatten_outer_dims()]
    for r in range(1, reps):
        scr = nc.dram_tensor(f"scratch{r}", [BL, T, D], DT).ap()
        out_fs.append(scr.flatten_outer_dims())

    def f32r(ap):
        return ap.bitcast(R)

    with tile.TileContext(nc) as tc:
        with (
            tc.tile_pool(name="consts", bufs=1) as consts,
            tc.tile_pool(name="y", bufs=1) as y_pool,
            tc.tile_pool(name="xt", bufs=2) as xt_pool,
            tc.tile_pool(name="qkv", bufs=1) as qkv_pool,
            tc.tile_pool(name="w", bufs=9) as w_pool,
            tc.tile_pool(name="xn", bufs=3) as xn_pool,
            tc.tile_pool(name="pp", bufs=12) as p_pool,
            tc.tile_pool(name="ytk", bufs=3) as ytk_pool,
            tc.tile_pool(name="ss", bufs=4) as s_pool,
            tc.tile_pool(name="ob", bufs=2) as o_pool,
        ):
            # ---------------- constants ----------------
            ident_f = consts.tile([PPART, PPART], DT)
            make_identity(nc, ident_f)
            ident = consts.tile([PPART, PPART], R)
            nc.vector.tensor_copy(out=ident, in_=ident_f)
            ident_b = consts.tile([PPART, PPART], BF)
            nc.vector.tensor_copy(out=ident_b, in_=ident_f)

            # mask for S^T [k-part, q-free]: keep q >= k, else -1e30
            maskneg = consts.tile([PPART, PPART], DT)
            nc.vector.memset(maskneg, 0.0)
            nc.gpsimd.affine_select(
                out=maskneg, in_=maskneg,
                compare_op=ALU.is_ge, fill=-1e30,
                base=0, pattern=[[1, PPART]], channel_multiplier=-1,
            )
            mask_bT = consts.tile([PPART, PPART], BF)
            nc.vector.tensor_copy(out=mask_bT, in_=maskneg)

            ones_f = consts.tile([1, PPART], DT)
            nc.vector.memset(ones_f, 1.0)
            ones_row = consts.tile([1, PPART], R)
            nc.vector.tensor_copy(out=ones_row, in_=ones_f)

            # biases: Q bias as [128, 8] columns (per-partition add on evac);
            # V bias as a [1, 1024] row (rank-1 matmul); output bias row.
            bq_col = consts.tile([PPART, 8], DT)
            bv_sb = consts.tile([1, D], R)
            bo_sb = consts.tile([1, D], R)
            bv_bc = consts.tile([PPART, D], BF)
            bo_bc = consts.tile([PPART, D], BF)

            def build_bias_bcast(qps):
                # broadcast V/O bias rows to all 128 partitions once, so the
                # per-group rank-1 bias matmuls disappear from the PE
                for dst, srow in ((bv_bc, bv_sb), (bo_bc, bo_sb)):
                    for half in range(2):
                        ps = qps.tile([PPART, T], DT, tag="ps", name="bbc")
                        nc.tensor.matmul(
                            ps, lhsT=ones_row,
                            rhs=srow[:, 512 * half:512 * (half + 1)],
                            start=True, stop=True)
                        nc.scalar.activation(
                            out=dst[:, 512 * half:512 * (half + 1)], in_=ps,
                            func=ACTF.Copy)

            def load_biases():
                nc.gpsimd.dma_start(
                    out=bq_col,
                    in_=bqkv_d.rearrange("(f p) -> p f", p=PPART)[:, 0:8])
                nc.gpsimd.dma_start(
                    out=bv_sb,
                    in_=f32r(bqkv_d.rearrange("(a f) -> a f", a=1))[:, 2 * D:3 * D])
                nc.gpsimd.dma_start(
                    out=bo_sb, in_=f32r(bo_d.rearrange("(a f) -> a f", a=1)))

            y_t = y_pool.tile([PPART, 8, BL * T], R)  # [128, 8, 1024] f32r

            def load_x(b, eng):
                xns = []
                for to in range(4):
                    xn = xn_pool.tile([PPART, D], R, tag="xn",
                                      name=f"xn{b}{to}")
                    rows = x_f[T * b + 128 * to:T * b + 128 * (to + 1), :]
                    if b == 0 and to == 0:
                        # split the very first load so fg0 transposes can
                        # start half a transfer earlier
                        eng.dma_start(out=xn[:, 0:512],
                                      in_=f32r(rows[:, 0:512]))
                        eng.dma_start(out=xn[:, 512:1024],
                                      in_=f32r(rows[:, 512:1024]))
                    else:
                        eng.dma_start(out=xn, in_=f32r(rows))
                    xns.append(xn)
                return xns

            def transpose_x(b, xns, trps, x_t):
                # to-major: each arriving dual-block is transposed (both
                # feature groups) while the next is still on the wire
                for to in range(4):
                    for fg in range(2):
                        pst = trps.tile([PPART, 4, PPART], R, tag="ps",
                                        name="pstx")
                        for fi in range(4):
                            fo = 4 * fg + fi
                            nc.tensor.transpose(
                                pst[:, fi, :],
                                xns[to][:, 128 * fo:128 * (fo + 1)],
                                ident)
                        if fg == 0:
                            nc.scalar.activation(
                                out=x_t[:, 4 * fg:4 * (fg + 1),
                                        128 * to:128 * (to + 1)],
                                in_=pst, func=ACTF.Copy)
                        else:
                            nc.vector.tensor_copy(
                                out=x_t[:, 4 * fg:4 * (fg + 1),
                                        128 * to:128 * (to + 1)],
                                in_=pst)

            # ---------------- QKV projection (fused over batches) ---------
            # q_t/k_t: [128(2 heads of pair j), pair j, T] bf16 feature-major
            # v_t: [128 tok, to, head, 65] bf16 token-major (+ones col)
            def stream_w_sec(sec):
                # quad-tiles: 2 DMAs per sec instead of 8 (SWDGE descriptor
                # generation on the queue is the scarce resource, not BW)
                wt = []
                for ko in range(8):
                    w_sb = w_pool.tile([PPART, 1024], R, tag="w",
                                       name=f"w{sec}_{ko}")
                    nc.sync.dma_start(
                        out=w_sb,
                        in_=f32r(wqkv_d[128 * ko:128 * (ko + 1),
                                        1024 * sec:1024 * (sec + 1)]))
                    wt.append(w_sb)
                return wt

            def qk_pass(sec, b, wt, x_t, qps, dst, fo_outer=False):
                # fo_outer staggers psum completions so the evac chain
                # pipelines with the matmuls instead of trailing the pass
                # (used for the last pass before the attention phase)
                psums = [qps.tile([PPART, T], DT, tag="ps",
                                  name=f"qkps{i}") for i in range(8)]
                if fo_outer:
                    for fo in range(8):
                        for ko in range(8):
                            nc.tensor.matmul(
                                psums[fo],
                                lhsT=wt[ko][:, 128 * fo:128 * (fo + 1)],
                                rhs=x_t[:, ko, :],
                                start=(ko == 0), stop=(ko == 7))
                else:
                    for ko in range(8):
                        for fo in range(8):
                            nc.tensor.matmul(
                                psums[fo],
                                lhsT=wt[ko][:, 128 * fo:128 * (fo + 1)],
                                rhs=x_t[:, ko, :],
                                start=(ko == 0), stop=(ko == 7))
                for fo in range(8):
                    if sec == 0:  # Q: add bias on evac (per-partition)
                        if fo % 2 == 0:
                            nc.scalar.activation(
                                out=dst[:, fo, :], in_=psums[fo],
                                func=ACTF.Identity,
                                bias=bq_col[:, fo:fo + 1])
                        else:
                            nc.vector.tensor_scalar_add(
                                out=dst[:, fo, :], in0=psums[fo],
                                scalar1=bq_col[:, fo:fo + 1])
                    else:         # K: bias dropped (softmax-invariant)
                        if fo % 2 == 0:
                            nc.scalar.activation(
                                out=dst[:, fo, :], in_=psums[fo],
                                func=ACTF.Copy)
                        else:
                            nc.vector.tensor_copy(
                                out=dst[:, fo, :], in_=psums[fo])

            wv_tiles = []

            def v_group(b, g, x_ts, v_ts, pool, tag="vps", evac="act"):
                to, nh = g // 2, g % 2
                ps = pool.tile([PPART, T], DT, tag=tag, name="vps")
                for ko in range(8):
                    nc.tensor.matmul(
                        ps,
                        lhsT=x_ts[b][:, ko, 128 * to:128 * (to + 1)],
                        rhs=wv_tiles[ko][:, 512 * nh:512 * (nh + 1)],
                        start=(ko == 0), stop=(ko == 7))
                nc.vector.tensor_add(
                    out=v_ts[b][:, to, 8 * nh:8 * (nh + 1), 0:64],
                    in0=ps, in1=bv_bc[:, 512 * nh:512 * (nh + 1)])

            # ---------------- attention (S^T scheme, bf16) ----------------
            def s_stage(b, h, q_ts, k_ts, sps):
                # kt=2 (256 cols) and kt=3 (128 cols) share one PSUM bank and
                # one fused exp op; returns [pch01 (kt0), pch1 (kt1), pch23]
                j, r = h // 2, h % 2
                base = 64 * r
                pchunks = []
                offs = [0, 0, 0, 256]   # col offset of chunk kt in its tile
                tiles = {}
                for kt in range(4):
                    cols = (4 - kt) * 128
                    if kt == 3:
                        ps = tiles[2]
                    else:
                        ps = sps.tile([PPART, T], DT, tag="s", name="sps")
                        tiles[kt] = ps
                    o = offs[kt]
                    nc.tensor.matmul(
                        ps[:, o:o + cols],
                        lhsT=k_ts[b][base:base + 64, j,
                                     128 * kt:128 * (kt + 1)],
                        rhs=q_ts[b][base:base + 64, j, 128 * kt:],
                        start=True, stop=True)
                for kt, cols in ((0, 512), (1, 384), (2, 384)):
                    pch = p_pool.tile([PPART, T], BF, tag="P", name="pch")
                    nc.scalar.activation(
                        out=pch[:, :cols], in_=tiles[kt][:, :cols],
                        func=ACTF.Exp, scale=1.0 / 32.0)
                    # causal mask: zero q < k on this chunk's diagonal
                    # block(s) on the (otherwise idle) GPSIMD engine
                    nc.gpsimd.affine_select(
                        out=pch[:, 0:128], in_=pch[:, 0:128],
                        compare_op=ALU.is_ge, fill=0.0,
                        base=0, pattern=[[1, PPART]], channel_multiplier=-1)
                    if kt == 2:  # kt=3's diagonal lives at offset 256
                        nc.gpsimd.affine_select(
                            out=pch[:, 256:384], in_=pch[:, 256:384],
                            compare_op=ALU.is_ge, fill=0.0,
                            base=0, pattern=[[1, PPART]],
                            channel_multiplier=-1)
                    pchunks.append(pch)
                return pchunks

            pair_state = {}

            def av_stage(b, h, pchunks, v_ts, yps, trps):
                j, r = h // 2, h % 2
                yp = yps.tile([PPART, 4, 65], DT, tag="y")
                for qt in range(4):
                    for kt in range(qt + 1):
                        pch = pchunks[min(kt, 2)]
                        o = 256 if kt == 3 else 0
                        nc.tensor.matmul(
                            yp[:, qt, :],
                            lhsT=pch[:, o + 128 * (qt - kt):
                                     o + 128 * (qt - kt + 1)],
                            rhs=v_ts[b][:, kt, h, :],
                            start=(kt == 0), stop=(kt == qt))
                rs = s_pool.tile([PPART, 4, 1], DT, tag="rs")
                nc.vector.reciprocal(rs, yp[:, :, 64:65])
                if r == 0:
                    ytk = ytk_pool.tile([PPART, 4, PPART], BF, tag="ytk")
                    pair_state[0] = ytk
                else:
                    ytk = pair_state[0]
                nc.vector.tensor_mul(
                    out=ytk[:, :, 64 * r:64 * (r + 1)],
                    in0=yp[:, :, 0:64],
                    in1=rs.broadcast_to([PPART, 4, 64]))
                if r == 1:  # pair complete: transpose back to feature-major
                    pst = trps.tile([PPART, 4, PPART], BF, tag="ytr")
                    for to in range(4):
                        nc.tensor.transpose(
                            pst[:, to, :], ytk[:, to, :], ident_b)
                    if b == 0 and j < 4:
                        nc.scalar.activation(
                            out=y_t[:, j, T * b:T * (b + 1)],
                            in_=pst, func=ACTF.Copy)
                    else:
                        nc.vector.tensor_copy(
                            out=y_t[:, j, T * b:T * (b + 1)], in_=pst)

            wo_tiles = {}

            def load_wo(wo_pool):
                for ko in range(8):
                    w_sb = wo_pool.tile([PPART, 1024], R, tag="w",
                                        name=f"wo{ko}")
                    nc.sync.dma_start(
                        out=w_sb, in_=f32r(wo_d[128 * ko:128 * (ko + 1), :]))
                    wo_tiles[ko] = w_sb

            wo_state = {}

            def wo_half(tg, nh, wps, out_f, split_dma=False):
                # both nh halves of a token group share one [128, 1024] evac
                # buffer and a single contiguous output DMA (issued on nh=1)
                if nh == 0:
                    ob = o_pool.tile([PPART, D], DT, tag="ob", name="ob")
                    wo_state[tg] = ob
                else:
                    ob = wo_state[tg]
                ps = wps.tile([PPART, T], DT, tag="s", name="wops")
                for ko in range(8):
                    nc.tensor.matmul(
                        ps,
                        lhsT=y_t[:, ko, 128 * tg:128 * (tg + 1)],
                        rhs=wo_tiles[ko][:, 512 * nh:512 * (nh + 1)],
                        start=(ko == 0), stop=(ko == 7))
                nc.vector.tensor_add(
                    out=ob[:, 512 * nh:512 * (nh + 1)], in0=ps,
                    in1=bo_bc[:, 512 * nh:512 * (nh + 1)])
                if nh == 1:
                    if split_dma:
                        for s in range(2):
                            q = nc.sync if s == 0 else nc.gpsimd
                            q.dma_start(
                                out=out_f[128 * tg:128 * (tg + 1),
                                          512 * s:512 * (s + 1)],
                                in_=ob[:, 512 * s:512 * (s + 1)])
                    else:
                        q = nc.sync if tg % 2 == 0 else nc.gpsimd
                        q.dma_start(
                            out=out_f[128 * tg:128 * (tg + 1), :], in_=ob)

            # ---------------- schedule ----------------
            for rep in range(reps):
                out_f = out_fs[rep]
                sfx = str(rep)
                wv_tiles.clear()
                wo_tiles.clear()

                x_t0 = xt_pool.tile([PPART, 8, T], R, tag="xt", name="xt0")
                x_t1 = xt_pool.tile([PPART, 8, T], R, tag="xt", name="xt1")
                x_ts = [x_t0, x_t1]

                q_ts = [qkv_pool.tile([PPART, 8, T], BF, tag=f"q{b}",
                                       name=f"q{b}")
                        for b in range(2)]
                k_ts = [qkv_pool.tile([PPART, 8, T], BF, tag=f"k{b}",
                                       name=f"k{b}")
                        for b in range(2)]
                v_ts = [qkv_pool.tile([PPART, 4, H, 65], BF, tag=f"v{b}",
                                       name=f"v{b}")
                        for b in range(2)]

                with tc.tile_pool(name="qps" + sfx, bufs=8,
                                  space="PSUM") as qps:
                    # x(b0) first on the fast SP queue, weights behind it;
                    # x(b1) + biases ride the Pool queue concurrently
                    xns0 = load_x(0, nc.sync)
                    wt0 = stream_w_sec(0)
                    if rep == 0:
                        load_biases()
                    xns1 = load_x(1, nc.sync)
                    transpose_x(0, xns0, qps, x_t0)
                    build_bias_bcast(qps)
                    qk_pass(0, 0, wt0, x_t0, qps, q_ts[0])
                    transpose_x(1, xns1, qps, x_t1)
                    wt1 = stream_w_sec(1)
                    qk_pass(0, 1, wt0, x_t1, qps, q_ts[1])
                    qk_pass(1, 0, wt1, x_t0, qps, k_ts[0])
                    wv_tiles.extend(stream_w_sec(2))
                    qk_pass(1, 1, wt1, x_t1, qps, k_ts[1])
                    for b in range(2):
                        nc.vector.memset(v_ts[b][:, :, :, 64:65], 1.0)

                # attn(b0): 2-head S-stage lookahead hides exp latency; ALL
                # V projection groups (b0 then b1) fill the PE between AV
                # stages, so attention (and its ACT exp stream) starts 14us
                # earlier. Groups a head's AV consumes must be emitted
                # before that AV (PE executes in order): nh=0 groups before
                # AV(0), nh=1 groups before AV(8).
                with (
                    tc.tile_pool(name="as0" + sfx, bufs=4, space="PSUM") as sps,
                    tc.tile_pool(name="ay0" + sfx, bufs=2, space="PSUM") as yps,
                    tc.tile_pool(name="at0" + sfx, bufs=2, space="PSUM") as trps,
                ):
                    order = ([(0, g) for g in (1, 3, 5, 7)]
                             + [(1, g) for g in range(8)])
                    vg = iter(order)
                    pend = []
                    for h in range(16):
                        pend.append((h, s_stage(0, h, q_ts, k_ts, sps)))
                        if h < 2:  # nh=0 groups of b0 up front
                            v_group(0, 4 * h + 0, x_ts, v_ts, sps, tag="s",
                                    evac="dve")
                            v_group(0, 4 * h + 2, x_ts, v_ts, sps, tag="s",
                                    evac="dve")
                        if len(pend) == 4:
                            hh, pch = pend.pop(0)
                            av_stage(0, hh, pch, v_ts, yps, trps)
                            bg = next(vg, None)
                            if bg is not None:
                                v_group(bg[0], bg[1], x_ts, v_ts, sps,
                                        tag="s", evac="dve")
                            if hh == 9:
                                load_wo(w_pool)
                    for hh, pch in pend:
                        av_stage(0, hh, pch, v_ts, yps, trps)
                        bg = next(vg, None)
                        if bg is not None:
                            v_group(bg[0], bg[1], x_ts, v_ts, sps,
                                    tag="s", evac="dve")

                    # attn(b1) interleaved with out-projection of b0 tokens
                    # (same pools: no PSUM handoff between the phases)
                    pend = []
                    wo_m = 0
                    for h in range(16):
                        pend.append((h, s_stage(1, h, q_ts, k_ts, sps)))
                        if len(pend) == 4:
                            hh, pch = pend.pop(0)
                            av_stage(1, hh, pch, v_ts, yps, trps)
                            if hh % 2 == 1:
                                wo_half(wo_m // 2, wo_m % 2, sps, out_f)
                                wo_m += 1
                    for hh, pch in pend:
                        av_stage(1, hh, pch, v_ts, yps, trps)
                    while wo_m < 8:
                        wo_half(wo_m // 2, wo_m % 2, sps, out_f)
                        wo_m += 1
                    # out-projection of b1 tokens; the last DMA is split
                    # across both queues so the tail drains faster
                    for tg in range(4, 8):
                        for nh in range(2):
                            wo_half(tg, nh, sps, out_f, split_dma=(tg == 7))

    nc.compile()
    return nc


def _get_program(reps=1, phases="xqaw"):
    key = f"nc{reps}{phases}"
    if key not in _CACHE:
        _CACHE[key] = _build_program(reps, phases)
    return _CACHE[key]


def kernel(x, w_qkv, b_qkv, w_o, b_o):
    from concourse.bass_utils import run_bass_kernel_spmd

    nc = _get_program()
    x = np.ascontiguousarray(x, dtype=np.float32)
    in_maps = []
    for c in range(NCORES):
        in_maps.append({
            "x": x[BL * c:BL * (c + 1)],
            "w_qkv": np.asarray(w_qkv, dtype=np.float32),
            "b_qkv": np.asarray(b_qkv, dtype=np.float32),
            "w_o": np.asarray(w_o, dtype=np.float32),
            "b_o": np.asarray(b_o, dtype=np.float32),
        })
    res = run_bass_kernel_spmd(nc, in_maps, list(range(NCORES)))
    return np.concatenate([res.results[c]["out"] for c in range(NCORES)], axis=0)
